# revision 30
# baseline (speedup 1.0000x reference)
"""AGNNConv (GNN message passing) Trainium2 Bass kernel, 8 NeuronCores.

Algorithm (reformulated from the reference, numerically equivalent):
  rnorm[n] = 1 / max(||feat[n]||, 1e-12)
  H[n]     = feat[n] * rnorm[n]                      (L2-normalized features)
  for edge e: w_e = exp(beta * <H[src_e], H[dst_e]>)
  out[n]   = (sum_{e: dst_e=n} w_e * feat[src_e]) / (sum_{e: dst_e=n} w_e)
The segment_max subtraction in the reference softmax is skipped: it cancels
exactly in the ratio, and |beta*cos| <= |beta| so exp never overflows.

Sharding: destination-node-tile ownership. Global dst tile g (128 nodes)
belongs to core g % 8 at position g // 8. Each core receives the edge lists
for its tiles (sorted/bucketed on host), gathers src/dst rows from a
device-built combined table [rnorm | H | norm] (bf16), computes edge weights,
and accumulates messages per dst tile with a one-hot matmul into PSUM.

The table is stored partition-major in DRAM: node n = g*128+p lives at
element (p * NT + g) * 130. Host precomputes element offsets for the
indirect gathers (coefficient=1), and prologue writes become a few large
contiguous-per-partition DMAs instead of 50k 260-byte descriptors.
"""

import math
import os

import numpy as np

import concourse.bass as bass
import concourse.mybir as mybir
from concourse.bass import IndirectOffsetOnAxis
from concourse.tile import TileContext

P = 128
D = 128
N_CORES = 8
TBL_COLS = D + 2  # [rnorm | H(128) | norm]
PRO_GROUP = 98  # prologue tiles per table-write DMA

F32 = mybir.dt.float32
BF16 = mybir.dt.bfloat16
I32 = mybir.dt.int32
NP_BF16 = mybir.dt.np(BF16)

AF = mybir.ActivationFunctionType
ALU = mybir.AluOpType


def _legalize_waits(nc):
    """This container's walrus rejects more than one embedded semaphore wait
    per engine instruction ("Too many sync wait commands"), and raw-ISA
    instructions (e.g. tensor_tensor_reduce) reject any embedded wait ("ISA
    wrong length"). Split extras into standalone EventSemaphore waits on the
    same engine right before the instruction — identical semantics (the
    sequencer blocks either way). Each spill carries a harmless dec on a
    dedicated dummy semaphore (walrus requires an update on EVSEM)."""
    import bass_rust

    dummy = nc.alloc_semaphore(name="legwait-dummy")
    ctr = [0]
    for f in nc.m.functions:
        for bb in f.blocks:
            lst = bb.instructions
            out = []
            changed = False
            for inst in lst:
                si = inst.sync_info
                tname = type(inst).__name__
                if tname == "InstEventSemaphore":
                    out.append(inst)
                    continue
                # raw-ISA-encoded instructions (custom DVE/Q7 ops) can't embed
                # any wait; standard engine instructions can embed exactly one
                ok_one = tname in (
                    "InstTensorTensor",
                    "InstActivation",
                    "InstMatmult",
                    "InstLdweights",
                    "InstTensorCopy",
                    "InstTensorScalarPtr",
                    "InstReciprocal",
                    "InstMemset",
                    "InstTensorReduce",
                    "InstDMACopy",
                    "InstDrain",
                    "InstIota",
                    "InstTensorScalarAffineSelect",
                )
                lim = 1 if ok_one else 0
                if si is not None and si.on_wait and len(si.on_wait) > lim:
                    waits = list(si.on_wait)
                    spill = waits[: len(waits) - lim]
                    for w in spill:
                        ev = mybir.InstEventSemaphore(
                            name=f"legwait-{ctr[0]}", ins=[], outs=[]
                        )
                        ctr[0] += 1
                        ev.engine = inst.engine
                        u = bass_rust.SyncUpdate(
                            sync_type="semaphore",
                            id=dummy.num,
                            ant_name="legwait-dummy",
                            update_mode="sem-inc",
                            update_value=1,
                        )
                        ev.sync_info = mybir.SyncInfo(on_wait=[w], on_update=[u])
                        out.append(ev)
                    si.on_wait = waits[len(waits) - lim :]
                    changed = True
                out.append(inst)
            if changed:
                bb.instructions = out


def build_graph(n_nodes, n_pos, k_list, legalize=True):
    """One SPMD graph shared by all cores. k_list[t] = #128-edge chunks for
    the core's t-th owned dst tile (same across cores by construction)."""
    sumk = sum(k_list)
    tot_idx = sumk * P
    nt = math.ceil(n_nodes / P)
    nc = bass.Bass()

    feat_ext = nc.declare_dram_parameter("feat", [n_nodes, D], F32, isOutput=False)
    beta_ext = nc.declare_dram_parameter("beta", [1, 1], F32, isOutput=False)
    soff_ext = nc.declare_dram_parameter("src_off", [tot_idx], I32, isOutput=False)
    doff_ext = nc.declare_dram_parameter("dst_off", [tot_idx], I32, isOutput=False)
    dstl_ext = nc.declare_dram_parameter("dstl", [tot_idx], BF16, isOutput=False)
    out_ext = nc.declare_dram_parameter("out", [n_pos * P, D], F32, isOutput=True)

    # partition-major table: element (p*nt + g)*TBL_COLS + c
    table = nc.dram_tensor("table", [P * nt * TBL_COLS], BF16)
    table_pm = table[:].rearrange("(p r) -> p r", p=P)

    with TileContext(nc) as tc:
        with (
            tc.tile_pool(name="const", bufs=1) as constp,
            tc.tile_pool(name="work", bufs=12) as work,
            tc.tile_pool(name="rowp", bufs=2) as rowp,
            tc.tile_pool(name="small", bufs=16) as small,
            tc.tile_pool(name="psum", bufs=6, space="PSUM") as psum,
        ):
            # build iota on DVE so one-hot compares never carry a DMA wait
            iota_i = constp.tile([P, P], I32)
            nc.gpsimd.iota(iota_i[:], pattern=[[1, P]], base=0, channel_multiplier=0)
            iota_t = constp.tile([P, P], BF16)
            nc.vector.tensor_copy(out=iota_t[:], in_=iota_i[:])
            beta_t = constp.tile([P, 1], F32)
            nc.sync.dma_start(out=beta_t[:], in_=beta_ext[:, :].to_broadcast((P, 1)))
            sidx_all = constp.tile([P, sumk], I32)
            nc.sync.dma_start(
                out=sidx_all[:], in_=soff_ext[:].rearrange("(p r) -> p r", p=P)
            )
            didx_all = constp.tile([P, sumk], I32)
            nc.sync.dma_start(
                out=didx_all[:], in_=doff_ext[:].rearrange("(p r) -> p r", p=P)
            )
            dstl_all = constp.tile([P, sumk], BF16)
            nc.sync.dma_start(
                out=dstl_all[:], in_=dstl_ext[:].rearrange("(p r) -> p r", p=P)
            )
            # funnel: advance DVE's clock past the dstl DMA once, so per-chunk
            # one-hot compares never embed a DMA wait (walrus sync-slot limit)
            jd = constp.tile([P, 1], BF16)
            nc.vector.tensor_copy(out=jd[:], in_=dstl_all[:, 0:1])

            # ---- prologue: combined table [rnorm | H | norm] (bf16) ----
            for g0 in range(0, nt, PRO_GROUP):
                ng = min(PRO_GROUP, nt - g0)
                rowbuf = rowp.tile([P, ng * TBL_COLS], BF16, tag="rowbuf")
                for gi in range(ng):
                    g = g0 + gi
                    r = min(P, n_nodes - g * P)
                    b = gi * TBL_COLS
                    if r < P:
                        nc.vector.memset(rowbuf[:, b : b + TBL_COLS], 0)
                    ft = work.tile([P, D], F32, tag="ft")
                    nc.sync.dma_start(out=ft[:r], in_=feat_ext[g * P : g * P + r, :])
                    sq = work.tile([P, D], F32, tag="sq")
                    ss = small.tile([P, 1], F32, tag="ss")
                    nc.scalar.activation(sq[:r], ft[:r], AF.Square, accum_out=ss[:r])
                    nrm = small.tile([P, 1], F32, tag="nrm")
                    nc.scalar.activation(nrm[:r], ss[:r], AF.Sqrt)
                    nrmc = small.tile([P, 1], F32, tag="nrmc")
                    nc.vector.tensor_scalar_max(nrmc[:r], nrm[:r], 1e-12)
                    rn = small.tile([P, 1], F32, tag="rn")
                    nc.vector.reciprocal(rn[:r], nrmc[:r])
                    nc.vector.tensor_tensor(
                        out=rowbuf[:r, b + 1 : b + 1 + D],
                        in0=ft[:r],
                        in1=rn[:r].to_broadcast((r, D)),
                        op=ALU.mult,
                    )
                    nc.scalar.activation(rowbuf[:r, b : b + 1], rn[:r], AF.Copy)
                    nc.scalar.activation(
                        rowbuf[:r, b + 1 + D : b + 2 + D], nrmc[:r], AF.Copy
                    )
                nc.sync.dma_start(
                    out=table_pm[:, g0 * TBL_COLS : (g0 + ng) * TBL_COLS],
                    in_=rowbuf[:],
                )

            # ---- main: per owned dst tile ----
            # NOTE: real HW honours only ONE index per partition per indirect
            # DMA (it reads contiguously from idx[p,0]), so gathers are issued
            # per 128-edge chunk.
            off = 0
            for t in range(n_pos):
                k = k_list[t]
                pt = psum.tile([P, 1 + D], F32)
                for j in range(k):
                    b = 0
                    hsr = work.tile([P, TBL_COLS], BF16, tag="hsr")
                    nc.gpsimd.indirect_dma_start(
                        out=hsr[:],
                        out_offset=None,
                        in_=table[:].rearrange("(r c) -> r c", c=1),
                        in_offset=IndirectOffsetOnAxis(
                            ap=sidx_all[:, off + j : off + j + 1], axis=0
                        ),
                    )
                    hd = work.tile([P, D], BF16, tag="hd")
                    nc.gpsimd.indirect_dma_start(
                        out=hd[:],
                        out_offset=None,
                        in_=table[:].rearrange("(r c) -> r c", c=1),
                        in_offset=IndirectOffsetOnAxis(
                            ap=didx_all[:, off + j : off + j + 1], axis=0
                        ),
                        element_offset=1,
                    )
                    prod = work.tile([P, D], BF16, tag="prod")
                    dot = small.tile([P, 1], F32, tag="dot")
                    nc.vector.tensor_tensor(
                        out=prod[:],
                        in0=hsr[:, b + 1 : b + 1 + D],
                        in1=hd[:],
                        op=ALU.mult,
                    )
                    nc.vector.reduce_sum(dot[:], prod[:], axis=mybir.AxisListType.X)
                    w = small.tile([P, 1], F32, tag="w")
                    nc.scalar.activation(w[:], dot[:], AF.Exp, scale=beta_t[:])
                    alpha = small.tile([P, 1], F32, tag="alpha")
                    nc.scalar.activation(
                        alpha[:], hsr[:, b + 1 + D : b + 2 + D], AF.Copy, scale=w[:]
                    )
                    rhs = work.tile([P, 1 + D], BF16, tag="rhs")
                    nc.scalar.activation(
                        rhs[:], hsr[:, b : b + 1 + D], AF.Copy, scale=alpha[:]
                    )
                    s_oh = work.tile([P, P], BF16, tag="s_oh")
                    nc.vector.tensor_tensor(
                        out=s_oh[:],
                        in0=dstl_all[:, off + j : off + j + 1].to_broadcast((P, P)),
                        in1=iota_t[:],
                        op=ALU.is_equal,
                    )
                    nc.tensor.matmul(
                        out=pt[:],
                        lhsT=s_oh[:],
                        rhs=rhs[:],
                        start=(j == 0),
                        stop=(j == k - 1),
                    )

                dmax = small.tile([P, 1], F32, tag="dmax")
                nc.vector.tensor_scalar_max(dmax[:], pt[:, 0:1], 1e-30)
                rec = small.tile([P, 1], F32, tag="rec")
                nc.vector.reciprocal(rec[:], dmax[:])
                ot = work.tile([P, D], F32, tag="ot")
                nc.vector.tensor_scalar_mul(ot[:], pt[:, 1 : 1 + D], rec[:])
                nc.sync.dma_start(out=out_ext[t * P : (t + 1) * P, :], in_=ot[:])
                off += k

    if legalize:
        _legalize_waits(nc)
    return nc


def shard_edges(src, dst, n_nodes, n_cores):
    """Bucket edges by dst tile; round-robin tile->core; pad each (core, pos)
    bucket to a shared chunk count. Returns per-core gather-offset arrays in
    global partition-major [128, sumk] layout."""
    nt = math.ceil(n_nodes / P)
    n_pos = math.ceil(nt / n_cores)
    g = dst // P
    order = np.argsort(g, kind="stable")
    g_sorted = g[order]
    starts = np.searchsorted(g_sorted, np.arange(nt + 1))

    counts = np.zeros((n_cores, n_pos), dtype=np.int64)
    for gg in range(nt):
        counts[gg % n_cores, gg // n_cores] = starts[gg + 1] - starts[gg]
    k_list = [max(1, int(math.ceil(counts[:, t].max() / P))) for t in range(n_pos)]
    sumk = sum(k_list)

    def to_off(node):
        return ((node % P) * nt + node // P) * TBL_COLS

    per_core = []
    for c in range(n_cores):
        soff = np.zeros((P, sumk), dtype=np.int32)
        doff = np.zeros((P, sumk), dtype=np.int32)
        dstl = np.full((P, sumk), -1.0, dtype=np.float32)
        col = 0
        for t in range(n_pos):
            k = k_list[t]
            gg = t * n_cores + c
            if gg < nt:
                e = order[starts[gg] : starts[gg + 1]]
                cnt = len(e)
                bs = np.zeros(k * P, dtype=np.int64)
                bd = np.zeros(k * P, dtype=np.int64)
                bl = np.full(k * P, -1.0, dtype=np.float32)
                bs[:cnt] = src[e]
                bd[:cnt] = dst[e]
                bl[:cnt] = (dst[e] - gg * P).astype(np.float32)
                soff[:, col : col + k] = to_off(bs).reshape(k, P).T
                doff[:, col : col + k] = to_off(bd).reshape(k, P).T
                dstl[:, col : col + k] = bl.reshape(k, P).T
            col += k
        per_core.append(
            {
                "src_off": soff.ravel(),
                "dst_off": doff.ravel(),
                "dstl": dstl.ravel().astype(NP_BF16),
            }
        )
    return n_pos, k_list, per_core


def _run_pjrt_timed(nc, in_maps, n_cores, time_iters=0):
    """run_bass_via_pjrt clone: zero output buffers passed as non-donated
    device-resident params (reusable), inputs device_put once, min-of-N
    timing."""
    import time

    import jax

    from concourse import bass2jax
    from concourse import mybir as mb
    from jax.sharding import Mesh, PartitionSpec
    from jax.experimental.shard_map import shard_map

    bass2jax.install_neuronx_cc_hook()

    part_name = nc.partition_id_tensor.name if nc.partition_id_tensor else None
    in_names, out_names, out_avals = [], [], []
    for alloc in nc.m.functions[0].allocations:
        if not isinstance(alloc, mb.MemoryLocationSet):
            continue
        name = alloc.memorylocations[0].name
        if alloc.kind == "ExternalInput":
            if name != part_name:
                in_names.append(name)
        elif alloc.kind == "ExternalOutput":
            out_names.append(name)
            out_avals.append(
                jax.core.ShapedArray(tuple(alloc.tensor_shape), mb.dt.np(alloc.dtype))
            )
    n_params = len(in_names)
    all_names = in_names + out_names
    if part_name is not None:
        all_names = all_names + [part_name]

    def _body(*args):
        operands = list(args)
        if part_name is not None:
            operands.append(bass2jax.partition_id_tensor())
        outs = bass2jax._bass_exec_p.bind(
            *operands,
            out_avals=tuple(out_avals),
            in_names=tuple(all_names),
            out_names=tuple(out_names),
            lowering_input_output_aliases=(),
            sim_require_finite=True,
            sim_require_nnan=True,
            nc=nc,
        )
        return tuple(outs)

    devices = jax.devices()[:n_cores]
    mesh = Mesh(np.asarray(devices), ("core",))
    sharded = jax.jit(
        shard_map(
            _body,
            mesh=mesh,
            in_specs=(PartitionSpec("core"),) * (n_params + len(out_names)),
            out_specs=(PartitionSpec("core"),) * len(out_names),
            check_rep=False,
        ),
        keep_unused=True,
    )
    concat_in = [
        jax.device_put(
            np.concatenate([np.asarray(in_maps[c][k]) for c in range(n_cores)], axis=0)
        )
        for k in in_names
    ] + [
        jax.device_put(np.zeros((n_cores * a.shape[0], *a.shape[1:]), a.dtype))
        for a in out_avals
    ]
    out_arrs = [np.asarray(o) for o in sharded(*concat_in)]

    if time_iters > 0:
        times = []
        for _ in range(time_iters):
            t0 = time.perf_counter()
            r = sharded(*concat_in)
            jax.block_until_ready(r)
            times.append(time.perf_counter() - t0)
        best = min(times)
        print(f"HW exec time: {best * 1e9:.0f} ns")
        print(f"wall times: {[f'{t*1e3:.2f}ms' for t in times]}")

    return [
        {
            name: out_arrs[i].reshape(n_cores, *out_avals[i].shape)[c]
            for i, name in enumerate(out_names)
        }
        for c in range(n_cores)
    ]


def kernel(feat, beta, src, dst):
    feat = np.asarray(feat, dtype=np.float32)
    beta = np.asarray(beta, dtype=np.float32)
    src = np.asarray(src, dtype=np.int32)
    dst = np.asarray(dst, dtype=np.int32)
    n_nodes = feat.shape[0]

    n_pos, k_list, per_core = shard_edges(src, dst, n_nodes, N_CORES)
    nc = build_graph(n_nodes, n_pos, k_list)

    in_maps = []
    for c in range(N_CORES):
        m = {
            "feat": feat,
            "beta": beta.reshape(1, 1),
            **per_core[c],
        }
        in_maps.append(m)

    iters = 5 if int(os.environ.get("BASS_KERNEL_TRACE", "0")) else 0
    results = _run_pjrt_timed(nc, in_maps, N_CORES, time_iters=iters)

    nt = math.ceil(n_nodes / P)
    out = np.zeros((nt * P, D), dtype=np.float32)
    for c in range(N_CORES):
        o = np.asarray(results[c]["out"])
        for t in range(n_pos):
            gg = t * N_CORES + c
            if gg < nt:
                out[gg * P : (gg + 1) * P] = o[t * P : (t + 1) * P]
    return out[:n_nodes]
